# revision 7
# baseline (speedup 1.0000x reference)
"""Trainium2 Bass kernel for ViT window attention with relative position bias.

Full inputs in, full outputs out. Data-parallel over batch: 64 batches split
8 per NeuronCore, weights replicated, no collectives.

All layout transforms (x/weight transposes, bf16 casts, rel-pos bias gather
and exp) happen on host in make_in_maps; the device graph is pure GEMMs plus
the softmax, so the tensor engine streams bf16 matmuls back to back.
"""

import os
import sys

for _p in ("/opt/trn_rl_repo", "/root/.axon_site/_ro/trn_rl_repo"):
    if os.path.isdir(_p) and _p not in sys.path:
        sys.path.insert(0, _p)

import numpy as np
import ml_dtypes

import concourse.bass as bass
import concourse.mybir as mybir
import concourse.tile as tile
from concourse import bacc
from concourse.bass import AP

F32 = mybir.dt.float32
BF16 = mybir.dt.bfloat16
AF = mybir.ActivationFunctionType

# problem constants
WIN = 14
NSEQ = WIN * WIN + 1          # 197
H = 16                        # heads
HD = 64                       # head dim
C = 1024
NREL = (2 * WIN - 1) * (2 * WIN - 1) + 3   # 732
B_FULL = 64
BC = 8                        # batches per core
T = BC * NSEQ                 # 1576 tokens per core
SCALE = HD ** -0.5            # 0.125
TCH = 394                     # qkv t-chunk (4 * 394 = 1576, fits one psum bank)
NT = 13                       # ceil(1576 / 128) output-projection token tiles
KA, KB = 128, 69              # key split per batch (197 = 128 + 69)
W2 = 2 * NSEQ                 # 394: two batches of queries per attention tile


def build_nc():
    nc = _build_graph()
    nc.compile()
    return nc


def _build_graph():
    nc = bacc.Bacc(None)

    xT_ext = nc.declare_dram_parameter("xT", [C, T], BF16, isOutput=False)
    wT_ext = nc.declare_dram_parameter("qkv_wT", [C, 3 * C], BF16, isOutput=False)
    pwT_ext = nc.declare_dram_parameter("proj_wT", [C, C], BF16, isOutput=False)
    eba_ext = nc.declare_dram_parameter("eba", [KA, H * W2], BF16, isOutput=False)
    ebb_ext = nc.declare_dram_parameter("ebb", [KB, H * W2], BF16, isOutput=False)
    qbc_ext = nc.declare_dram_parameter("qb_col", [128, 8], F32, isOutput=False)
    vbb_ext = nc.declare_dram_parameter("vb_bcast", [128, C], BF16, isOutput=False)
    pbb_ext = nc.declare_dram_parameter("pb_bcast", [128, C], F32, isOutput=False)
    out_ext = nc.declare_dram_parameter("out", [T, C], F32, isOutput=True)

    with tile.TileContext(nc) as tc:
        with tc.tile_pool(name="persist", bufs=1) as pp:
            onesK = pp.tile([128, HD], BF16, name="onesK", tag="onesK")
            nc.gpsimd.memset(onesK[:], 1.0)
            qb_col = pp.tile([128, 8], F32, name="qb_col", tag="qb_col")
            nc.sync.dma_start(qb_col[:], qbc_ext[:])
            vbb = pp.tile([128, C], BF16, name="vbb", tag="vbb")
            pbb = pp.tile([128, C], F32, name="pbb", tag="pbb")
            EBA = pp.tile([KA, H * W2], BF16, name="eba", tag="eba")
            EBB = pp.tile([KB, H * W2], BF16, name="ebb", tag="ebb")

            # persistent products of phase 1
            QKT = [
                pp.tile([128, T], BF16, name=f"qkt{ot}", tag=f"qkt{ot}")
                for ot in range(16)
            ]
            V1 = {}
            for b in range(BC):
                V1[(b, 0)] = pp.tile([KA, C], BF16, name=f"v1_{b}_0", tag=f"v1_{b}_0")
                V1[(b, 1)] = pp.tile([KB, C], BF16, name=f"v1_{b}_1", tag=f"v1_{b}_1")

            # ----- phase 1: QKV projection ---------------------------------
            with (
                tc.tile_pool(name="xw", bufs=1) as xw,
                tc.tile_pool(name="ps1", bufs=4, space="PSUM") as ps1,
            ):
                # Block-nested SBUF layouts: each DMA fills a contiguous,
                # disjoint column range so matmuls only wait on the one DMA
                # that wrote their operand (region deps stay tight).
                # xTall col(ct, t) = tci*3152 + ct*394 + t%394, tci = t//394
                # WTall col(ct, o) = og*4096 + ct*512 + o%512,  og = o//512
                xTall = xw.tile([128, 8 * T], BF16, name="xtall", tag="xtall")
                WTall = xw.tile([128, 8 * 3 * C], BF16, name="wtall", tag="wtall")

                def xs(ct, t0, tl):  # x^T slice: chunk ct, tokens [t0, t0+tl)
                    tci = t0 // TCH
                    assert (t0 - tci * TCH) + tl <= TCH
                    a = xTall[:]
                    return AP(a.tensor,
                              a.offset + tci * 8 * TCH + ct * TCH + t0 % TCH,
                              [a.ap[0], [1, tl]])

                def ws(ct, o0, ol):  # qkv_w^T slice: chunk ct, outputs o range
                    og = o0 // 512
                    assert (o0 - og * 512) + ol <= 512
                    a = WTall[:]
                    return AP(a.tensor,
                              a.offset + og * 4096 + ct * 512 + o0 % 512,
                              [a.ap[0], [1, ol]])

                xa = xTall[:]
                wa = WTall[:]

                def dma_stripe(eng, tci, c0, cn):
                    eng.dma_start(
                        AP(xa.tensor, xa.offset + tci * 8 * TCH + c0 * TCH,
                           [xa.ap[0], [1, cn * TCH]]),
                        AP(xT_ext, c0 * 128 * T + tci * TCH,
                           [[T, 128], [128 * T, cn], [1, TCH]]),
                    )

                def dma_wog(eng, og, c0, cn):
                    eng.dma_start(
                        AP(wa.tensor, wa.offset + og * 4096 + c0 * 512,
                           [wa.ap[0], [1, cn * 512]]),
                        AP(wT_ext, c0 * 128 * 3 * C + og * 512,
                           [[3 * C, 128], [128 * 3 * C, cn], [1, 512]]),
                    )

                # cold start: first stripe and o-groups split in halves,
                # spread across the three DMA-issuing queues
                dma_stripe(nc.sync, 0, 0, 4)
                dma_wog(nc.scalar, 0, 0, 4)
                dma_stripe(nc.sync, 0, 4, 4)
                dma_wog(nc.scalar, 0, 4, 4)
                dma_wog(nc.gpsimd, 2, 0, 8)
                for tci in range(1, 4):
                    dma_stripe(nc.sync, tci, 0, 8)
                dma_wog(nc.scalar, 1, 0, 8)
                dma_wog(nc.gpsimd, 3, 0, 8)
                nc.gpsimd.dma_start(EBA[:], eba_ext[:])
                nc.gpsimd.dma_start(EBB[:], ebb_ext[:])
                dma_wog(nc.scalar, 4, 0, 8)
                dma_wog(nc.scalar, 5, 0, 8)
                nc.sync.dma_start(vbb[:], vbb_ext[:])
                nc.sync.dma_start(pbb[:], pbb_ext[:])

                def qk_chunk(ot, tci):
                    t0 = tci * TCH
                    pq = ps1.tile([128, 512], F32, name="pq", tag="pq")
                    for ct in range(8):
                        nc.tensor.matmul(
                            pq[:, 0:TCH],
                            ws(ct, ot * 128, 128),
                            xs(ct, t0, TCH),
                            start=(ct == 0), stop=(ct == 7),
                        )
                    if ot < 8:
                        nc.scalar.add(
                            QKT[ot][:, t0:t0 + TCH], pq[:, 0:TCH],
                            qb_col[:, ot:ot + 1],
                        )
                    else:
                        nc.vector.tensor_copy(QKT[ot][:, t0:t0 + TCH], pq[:, 0:TCH])

                # tci-outer matches the DMA arrival order (stripe k lands
                # while the previous stripe's 16 o-tiles stream).
                for tci in range(4):
                    for ot in range(16):
                        qk_chunk(ot, tci)

                # V projection -> V1[(b,kt)] bf16 (+bias)
                for b in range(BC):
                    for kt, (ko, ksz) in enumerate(((0, KA), (KA, KB))):
                        t0 = b * NSEQ + ko
                        for oc in range(2):
                            pv = ps1.tile([128, 512], F32, name="pv", tag="pq")
                            for ct in range(8):
                                nc.tensor.matmul(
                                    pv[0:ksz, :],
                                    xs(ct, t0, ksz),
                                    ws(ct, 2 * C + oc * 512, 512),
                                    start=(ct == 0), stop=(ct == 7),
                                )
                            nc.vector.tensor_add(
                                V1[(b, kt)][0:ksz, oc * 512:(oc + 1) * 512],
                                pv[0:ksz, :],
                                vbb[0:ksz, oc * 512:(oc + 1) * 512],
                            )

            # ----- phase 2: attention + output projection -------------------
            with (
                tc.tile_pool(name="ab", bufs=1) as ab,
                tc.tile_pool(name="attw", bufs=8) as attw,
                tc.tile_pool(name="attsmall", bufs=6) as attsmall,
                tc.tile_pool(name="ostage", bufs=3) as op_,
                tc.tile_pool(name="ps_s", bufs=2, space="PSUM") as ps_s,
                tc.tile_pool(name="ps_o", bufs=2, space="PSUM") as ps_o,
                tc.tile_pool(name="ps_r", bufs=2, space="PSUM") as ps_r,
            ):
                attnT = [
                    ab.tile([128, T], BF16, name=f"at{ct}", tag=f"at{ct}")
                    for ct in range(8)
                ]
                # PWTall col(ct, o) = oc*4096 + ct*512 + o%512, oc = o//512
                PWTall = ab.tile([128, 8 * C], BF16, name="pwtall", tag="pwtall")
                pa = PWTall[:]
                for oc in range(2):
                    nc.sync.dma_start(
                        AP(pa.tensor, pa.offset + oc * 4096,
                           [pa.ap[0], [1, 4096]]),
                        AP(pwT_ext, oc * 512,
                           [[C, 128], [128 * C, 8], [1, 512]]),
                    )

                def pws(ct, o0, ol):
                    oc = o0 // 512
                    assert (o0 - oc * 512) + ol <= 512
                    a = PWTall[:]
                    return AP(a.tensor,
                              a.offset + oc * 4096 + ct * 512 + o0 % 512,
                              [a.ap[0], [1, ol]])

                # output projection, emitted one 512-col half at a time so it
                # can fill tensor-engine gaps inside the attention loop
                b4_state = {"tt": 0, "oc": 0, "ost": None}

                def b4_ready():
                    return b4_state["tt"] < NT

                def emit_b4_half(limit_tt):
                    tt, oc = b4_state["tt"], b4_state["oc"]
                    if tt >= min(NT, limit_tt):
                        return False
                    tsz = min(128, T - tt * 128)
                    if oc == 0:
                        b4_state["ost"] = op_.tile([128, C], F32, name="ost",
                                                   tag="ost")
                    ost = b4_state["ost"]
                    pp2 = ps_r.tile([128, 512], F32, name="ppj", tag="rbp")
                    for ct in range(8):
                        nc.tensor.matmul(
                            pp2[0:tsz, :],
                            attnT[ct][:, tt * 128:tt * 128 + tsz],
                            pws(ct, oc * 512, 512),
                            start=(ct == 0), stop=(ct == 7),
                        )
                    nc.vector.tensor_add(
                        ost[0:tsz, oc * 512:(oc + 1) * 512],
                        pp2[0:tsz, :],
                        pbb[0:tsz, oc * 512:(oc + 1) * 512],
                    )
                    if oc == 1:
                        nc.sync.dma_start(
                            out_ext[tt * 128:tt * 128 + tsz, :], ost[0:tsz, :]
                        )
                        b4_state["tt"] += 1
                        b4_state["oc"] = 0
                    else:
                        b4_state["oc"] = 1
                    return True

                for bp in range(BC // 2):
                    t0p = 2 * bp * NSEQ
                    # attnT token range complete for batches < 2*bp
                    ready_tt = (2 * bp * NSEQ) // 128
                    for hp in range(8):
                        emit_b4_half(ready_tt)
                        po = ps_o.tile([128, W2], F32, name="po",
                                       tag="po", padded_shape=[128, 512])
                        kot = 8 + hp
                        pss, pts = [], []
                        for hh in range(2):
                            qpo = 64 * hh
                            ps = ps_s.tile([128, 1024], F32, name="ps",
                                           tag="ps")
                            for kt, (ko, ksz, co) in enumerate(
                                    ((0, KA, 0), (KA, KB, 512))):
                                for bi in range(2):
                                    t0 = (2 * bp + bi) * NSEQ
                                    nc.tensor.matmul(
                                        ps[0:ksz,
                                           co + bi * NSEQ:co + (bi + 1) * NSEQ],
                                        QKT[kot][qpo:qpo + 64,
                                                 t0 + ko:t0 + ko + ksz],
                                        QKT[hp][qpo:qpo + 64, t0:t0 + NSEQ],
                                        start=True, stop=True,
                                    )
                            pss.append(ps)
                        emit_b4_half(ready_tt)
                        rbp = ps_r.tile([128, W2], F32, name="rbp",
                                        tag="rbp", padded_shape=[128, 512])
                        for hh in range(2):
                            h = 2 * hp + hh
                            cpos = 64 * hh
                            ps = pss[hh]
                            pt = attw.tile([128, 1024], BF16, name="pt",
                                           tag="pt")
                            nc.scalar.activation(
                                pt[:], ps[:], AF.Exp, scale=SCALE,
                            )
                            nc.vector.tensor_mul(
                                pt[0:KA, 0:W2],
                                pt[0:KA, 0:W2],
                                EBA[:, h * W2:(h + 1) * W2],
                            )
                            nc.gpsimd.tensor_mul(
                                pt[0:KB, 512:512 + W2],
                                pt[0:KB, 512:512 + W2],
                                EBB[:, h * W2:(h + 1) * W2],
                            )
                            pts.append(pt)
                            for kt, (ko, ksz, co) in enumerate(
                                    ((0, KA, 0), (KA, KB, 512))):
                                nc.tensor.matmul(
                                    rbp[cpos:cpos + 64, :],
                                    onesK[0:ksz, :], pt[0:ksz, co:co + W2],
                                    start=(kt == 0), stop=(kt == 1),
                                    tile_position=(0, cpos),
                                )
                            for bi in range(2):
                                b = 2 * bp + bi
                                for kt, (ko, ksz, co) in enumerate(
                                        ((0, KA, 0), (KA, KB, 512))):
                                    nc.tensor.matmul(
                                        po[cpos:cpos + 64,
                                           bi * NSEQ:(bi + 1) * NSEQ],
                                        V1[(b, kt)][0:ksz, h * HD:(h + 1) * HD],
                                        pt[0:ksz,
                                           co + bi * NSEQ:co + (bi + 1) * NSEQ],
                                        start=(kt == 0), stop=(kt == 1),
                                        tile_position=(0, cpos),
                                    )
                        rbs = attsmall.tile([128, W2], F32,
                                            name="rbs", tag="rbs")
                        nc.vector.reciprocal_approx_fast(rbs[:], rbp[:])
                        nc.vector.tensor_mul(
                            attnT[hp][:, t0p:t0p + W2], po[:], rbs[:]
                        )
                while emit_b4_half(NT):
                    pass

    return nc


_NC = None
LAST_RESULT = None


def _get_nc():
    global _NC
    if _NC is None:
        _NC = build_nc()
    return _NC


def _default_rel_pos_index():
    coords = np.stack(np.meshgrid(np.arange(WIN), np.arange(WIN), indexing='ij'))
    coords_flatten = coords.reshape(2, -1)
    rel = coords_flatten[:, :, None] - coords_flatten[:, None, :]
    rel = rel.transpose(1, 2, 0).astype(np.int64)
    rel[:, :, 0] += WIN - 1
    rel[:, :, 1] += WIN - 1
    rel[:, :, 0] *= 2 * WIN - 1
    idx = np.zeros((NSEQ, NSEQ), dtype=np.int64)
    idx[1:, 1:] = rel.sum(-1)
    idx[0, :] = NREL - 3
    idx[:, 0] = NREL - 2
    idx[0, 0] = NREL - 1
    return idx


def make_in_maps(x, qkv_w, q_bias, v_bias, rpb_table, proj_w, proj_b,
                 rel_pos_index=None):
    bf = ml_dtypes.bfloat16
    x = np.asarray(x, np.float32)
    wT = np.ascontiguousarray(np.asarray(qkv_w, np.float32).T.astype(bf))
    pwT = np.ascontiguousarray(np.asarray(proj_w, np.float32).T.astype(bf))
    qbc = np.ascontiguousarray(
        np.asarray(q_bias, np.float32).reshape(8, 128).T
    )
    vbb = np.ascontiguousarray(
        np.broadcast_to(np.asarray(v_bias, np.float32), (128, C)).astype(bf)
    )
    pbb = np.ascontiguousarray(
        np.broadcast_to(np.asarray(proj_b, np.float32), (128, C))
    )
    idx = (np.asarray(rel_pos_index) if rel_pos_index is not None
           else _default_rel_pos_index())
    bias = np.asarray(rpb_table, np.float32)[idx]          # [q, k, h]
    ebT = np.exp(bias).transpose(2, 1, 0)                  # [h, k, q]
    eb2 = np.concatenate([ebT, ebT], axis=2)               # [h, k, 2 batches q]
    eba = np.ascontiguousarray(
        eb2[:, :KA, :].transpose(1, 0, 2).reshape(KA, H * W2).astype(bf)
    )
    ebb = np.ascontiguousarray(
        eb2[:, KA:, :].transpose(1, 0, 2).reshape(KB, H * W2).astype(bf)
    )
    in_maps = []
    for c in range(8):
        xs = np.ascontiguousarray(
            x[c * BC:(c + 1) * BC].reshape(T, C).T.astype(bf)
        )
        in_maps.append({
            "xT": xs, "qkv_wT": wT, "proj_wT": pwT, "eba": eba, "ebb": ebb,
            "qb_col": qbc, "vb_bcast": vbb, "pb_bcast": pbb,
        })
    return in_maps


def _ensure_axon_hooks_importable():
    """bass_utils imports antenv.axon_hooks when BASS_TRACE is set; the image's
    antenv lacks that module. Provide a no-op stand-in so tracing degrades
    gracefully instead of crashing (unless a real one is already installed)."""
    import types
    try:
        import antenv.axon_hooks  # noqa: F401
    except Exception:
        mod = types.ModuleType("antenv.axon_hooks")
        mod._h = None
        mod.set_axon_ntff_profile_hook = lambda h: setattr(mod, "_h", h)
        mod.get_axon_ntff_profile_hook = lambda: mod._h
        sys.modules["antenv.axon_hooks"] = mod
        try:
            import antenv
            antenv.axon_hooks = mod
        except Exception:
            pass


def kernel(x, qkv_w, q_bias, v_bias, rpb_table, proj_w, proj_b,
           rel_pos_index=None, **_unused):
    global LAST_RESULT
    _ensure_axon_hooks_importable()
    from concourse.bass_utils import run_bass_kernel_spmd

    nc = _get_nc()
    in_maps = make_in_maps(x, qkv_w, q_bias, v_bias, rpb_table, proj_w, proj_b,
                           rel_pos_index)
    res = run_bass_kernel_spmd(nc, in_maps, core_ids=list(range(8)))
    LAST_RESULT = res
    out = np.concatenate(
        [res.results[c]["out"].reshape(BC, NSEQ, C) for c in range(8)], axis=0
    )
    return out.astype(np.float32)


# revision 9
# speedup vs baseline: 1.1204x; 1.1204x over previous
"""Trainium2 Bass kernel for ViT window attention with relative position bias.

Full inputs in, full outputs out. Data-parallel over batch: 64 batches split
8 per NeuronCore, weights replicated, no collectives.

All layout transforms (x/weight transposes, bf16 casts, rel-pos bias gather
and exp) happen on host in make_in_maps; the device graph is pure GEMMs plus
the softmax, so the tensor engine streams bf16 matmuls back to back.
"""

import os
import sys

for _p in ("/opt/trn_rl_repo", "/root/.axon_site/_ro/trn_rl_repo"):
    if os.path.isdir(_p) and _p not in sys.path:
        sys.path.insert(0, _p)

import numpy as np
import ml_dtypes

import concourse.bass as bass
import concourse.mybir as mybir
import concourse.tile as tile
from concourse import bacc
from concourse.bass import AP

F32 = mybir.dt.float32
BF16 = mybir.dt.bfloat16
AF = mybir.ActivationFunctionType

# problem constants
WIN = 14
NSEQ = WIN * WIN + 1          # 197
H = 16                        # heads
HD = 64                       # head dim
C = 1024
NREL = (2 * WIN - 1) * (2 * WIN - 1) + 3   # 732
B_FULL = 64
BC = 8                        # batches per core
T = BC * NSEQ                 # 1576 tokens per core
SCALE = HD ** -0.5            # 0.125
TCH = 394                     # qkv t-chunk (4 * 394 = 1576, fits one psum bank)
NT = 13                       # ceil(1576 / 128) output-projection token tiles
KA, KB = 128, 69              # key split per batch (197 = 128 + 69)
W2 = 2 * NSEQ                 # 394: two batches of queries per attention tile


def build_nc():
    nc = _build_graph()
    nc.compile()
    return nc


def _build_graph():
    nc = bacc.Bacc(None)

    xT_ext = nc.declare_dram_parameter("xT", [C, T], BF16, isOutput=False)
    wT_ext = nc.declare_dram_parameter("qkv_wT", [C, 3 * C], BF16, isOutput=False)
    pwT_ext = nc.declare_dram_parameter("proj_wT", [C, C], BF16, isOutput=False)
    eba_ext = nc.declare_dram_parameter("eba", [KA, H * W2], BF16, isOutput=False)
    ebb_ext = nc.declare_dram_parameter("ebb", [KB, H * W2], BF16, isOutput=False)
    qbc_ext = nc.declare_dram_parameter("qb_col", [128, 8], F32, isOutput=False)
    vbb_ext = nc.declare_dram_parameter("vb_bcast", [128, C], BF16, isOutput=False)
    pbb_ext = nc.declare_dram_parameter("pb_bcast", [128, C], F32, isOutput=False)
    out_ext = nc.declare_dram_parameter("out", [T, C], F32, isOutput=True)

    with tile.TileContext(nc) as tc:
        with tc.tile_pool(name="persist", bufs=1) as pp:
            onesK = pp.tile([128, HD], BF16, name="onesK", tag="onesK")
            nc.gpsimd.memset(onesK[:], 1.0)
            qb_col = pp.tile([128, 8], F32, name="qb_col", tag="qb_col")
            nc.sync.dma_start(qb_col[:], qbc_ext[:])
            vbb = pp.tile([128, C], BF16, name="vbb", tag="vbb")
            pbb = pp.tile([128, C], F32, name="pbb", tag="pbb")
            EBA = pp.tile([KA, H * W2], BF16, name="eba", tag="eba")
            EBB = pp.tile([KB, H * W2], BF16, name="ebb", tag="ebb")

            # persistent products of phase 1
            QKT = [
                pp.tile([128, T], BF16, name=f"qkt{ot}", tag=f"qkt{ot}")
                for ot in range(16)
            ]
            V1 = {}
            for b in range(BC):
                V1[(b, 0)] = pp.tile([KA, C], BF16, name=f"v1_{b}_0", tag=f"v1_{b}_0")
                V1[(b, 1)] = pp.tile([KB, C], BF16, name=f"v1_{b}_1", tag=f"v1_{b}_1")

            # ----- phase 1: QKV projection ---------------------------------
            with (
                tc.tile_pool(name="xw", bufs=1) as xw,
                tc.tile_pool(name="ps1", bufs=4, space="PSUM") as ps1,
            ):
                # Block-nested SBUF layouts: each DMA fills a contiguous,
                # disjoint column range so matmuls only wait on the one DMA
                # that wrote their operand (region deps stay tight).
                # xTall col(ct, t) = tci*3152 + ct*394 + t%394, tci = t//394
                # WTall col(ct, o) = og*4096 + ct*512 + o%512,  og = o//512
                xTall = xw.tile([128, 8 * T], BF16, name="xtall", tag="xtall")
                WTall = xw.tile([128, 8 * 3 * C], BF16, name="wtall", tag="wtall")

                def xs(ct, t0, tl):  # x^T slice: chunk ct, tokens [t0, t0+tl)
                    tci = t0 // TCH
                    assert (t0 - tci * TCH) + tl <= TCH
                    a = xTall[:]
                    return AP(a.tensor,
                              a.offset + tci * 8 * TCH + ct * TCH + t0 % TCH,
                              [a.ap[0], [1, tl]])

                def ws(ct, o0, ol):  # qkv_w^T slice: chunk ct, outputs o range
                    og = o0 // 512
                    assert (o0 - og * 512) + ol <= 512
                    a = WTall[:]
                    return AP(a.tensor,
                              a.offset + og * 4096 + ct * 512 + o0 % 512,
                              [a.ap[0], [1, ol]])

                xa = xTall[:]
                wa = WTall[:]

                def dma_stripe(eng, tci, c0, cn):
                    eng.dma_start(
                        AP(xa.tensor, xa.offset + tci * 8 * TCH + c0 * TCH,
                           [xa.ap[0], [1, cn * TCH]]),
                        AP(xT_ext, c0 * 128 * T + tci * TCH,
                           [[T, 128], [128 * T, cn], [1, TCH]]),
                    )

                def dma_wog(eng, og, c0, cn):
                    eng.dma_start(
                        AP(wa.tensor, wa.offset + og * 4096 + c0 * 512,
                           [wa.ap[0], [1, cn * 512]]),
                        AP(wT_ext, c0 * 128 * 3 * C + og * 512,
                           [[3 * C, 128], [128 * 3 * C, cn], [1, 512]]),
                    )

                # cold start: spread across the three DMA-issuing queues in
                # the order the anti-diagonal QK schedule consumes the blocks
                dma_stripe(nc.sync, 0, 0, 4)
                dma_wog(nc.scalar, 0, 0, 4)
                dma_wog(nc.gpsimd, 1, 4, 4)
                dma_stripe(nc.sync, 0, 4, 4)
                dma_wog(nc.scalar, 0, 4, 4)
                dma_stripe(nc.sync, 1, 0, 8)
                dma_wog(nc.scalar, 1, 0, 4)
                dma_wog(nc.gpsimd, 2, 0, 8)
                dma_stripe(nc.sync, 2, 0, 8)
                dma_wog(nc.sync, 3, 0, 8)
                dma_stripe(nc.sync, 3, 0, 8)
                dma_wog(nc.scalar, 4, 0, 8)
                dma_wog(nc.scalar, 5, 0, 8)
                nc.gpsimd.dma_start(EBA[:], eba_ext[:])
                nc.gpsimd.dma_start(EBB[:], ebb_ext[:])
                nc.sync.dma_start(vbb[:], vbb_ext[:])
                nc.sync.dma_start(pbb[:], pbb_ext[:])

                def qk_chunk(ot, tci):
                    t0 = tci * TCH
                    pq = ps1.tile([128, 512], F32, name="pq", tag="pq")
                    for ct in range(8):
                        nc.tensor.matmul(
                            pq[:, 0:TCH],
                            ws(ct, ot * 128, 128),
                            xs(ct, t0, TCH),
                            start=(ct == 0), stop=(ct == 7),
                        )
                    if ot < 8:
                        nc.scalar.add(
                            QKT[ot][:, t0:t0 + TCH], pq[:, 0:TCH],
                            qb_col[:, ot:ot + 1],
                        )
                    else:
                        nc.vector.tensor_copy(QKT[ot][:, t0:t0 + TCH], pq[:, 0:TCH])

                # anti-diagonal (og, stripe) order matches DMA arrivals
                diag = sorted(
                    ((og, tci) for og in range(4) for tci in range(4)),
                    key=lambda p: (p[0] + p[1], p[1]),
                )
                for og, tci in diag:
                    for ot in range(4 * og, 4 * og + 4):
                        qk_chunk(ot, tci)

                # V projection -> V1[(b,kt)] bf16 (+bias)
                for b in range(BC):
                    for kt, (ko, ksz) in enumerate(((0, KA), (KA, KB))):
                        t0 = b * NSEQ + ko
                        for oc in range(2):
                            pv = ps1.tile([128, 512], F32, name="pv", tag="pq")
                            for ct in range(8):
                                nc.tensor.matmul(
                                    pv[0:ksz, :],
                                    xs(ct, t0, ksz),
                                    ws(ct, 2 * C + oc * 512, 512),
                                    start=(ct == 0), stop=(ct == 7),
                                )
                            nc.vector.tensor_add(
                                V1[(b, kt)][0:ksz, oc * 512:(oc + 1) * 512],
                                pv[0:ksz, :],
                                vbb[0:ksz, oc * 512:(oc + 1) * 512],
                            )

            # ----- phase 2: attention + output projection -------------------
            with (
                tc.tile_pool(name="ab", bufs=1) as ab,
                tc.tile_pool(name="attw", bufs=8) as attw,
                tc.tile_pool(name="attsmall", bufs=6) as attsmall,
                tc.tile_pool(name="ostage", bufs=3) as op_,
                tc.tile_pool(name="ps_s", bufs=2, space="PSUM") as ps_s,
                tc.tile_pool(name="ps_o", bufs=2, space="PSUM") as ps_o,
                tc.tile_pool(name="ps_r", bufs=2, space="PSUM") as ps_r,
            ):
                attnT = [
                    ab.tile([128, T], BF16, name=f"at{ct}", tag=f"at{ct}")
                    for ct in range(8)
                ]
                # PWTall col(ct, o) = oc*4096 + ct*512 + o%512, oc = o//512
                PWTall = ab.tile([128, 8 * C], BF16, name="pwtall", tag="pwtall")
                pa = PWTall[:]
                for oc in range(2):
                    nc.sync.dma_start(
                        AP(pa.tensor, pa.offset + oc * 4096,
                           [pa.ap[0], [1, 4096]]),
                        AP(pwT_ext, oc * 512,
                           [[C, 128], [128 * C, 8], [1, 512]]),
                    )

                def pws(ct, o0, ol):
                    oc = o0 // 512
                    assert (o0 - oc * 512) + ol <= 512
                    a = PWTall[:]
                    return AP(a.tensor,
                              a.offset + oc * 4096 + ct * 512 + o0 % 512,
                              [a.ap[0], [1, ol]])

                # output projection, emitted one 512-col half at a time so it
                # can fill tensor-engine gaps inside the attention loop
                b4_state = {"tt": 0, "oc": 0, "ost": None}

                def b4_ready():
                    return b4_state["tt"] < NT

                def emit_b4_half(limit_tt):
                    tt, oc = b4_state["tt"], b4_state["oc"]
                    if tt >= min(NT, limit_tt):
                        return False
                    tsz = min(128, T - tt * 128)
                    if oc == 0:
                        b4_state["ost"] = op_.tile([128, C], F32, name="ost",
                                                   tag="ost")
                    ost = b4_state["ost"]
                    pp2 = ps_r.tile([128, 512], F32, name="ppj", tag="rbp")
                    for ct in range(8):
                        nc.tensor.matmul(
                            pp2[0:tsz, :],
                            attnT[ct][:, tt * 128:tt * 128 + tsz],
                            pws(ct, oc * 512, 512),
                            start=(ct == 0), stop=(ct == 7),
                        )
                    nc.vector.tensor_add(
                        ost[0:tsz, oc * 512:(oc + 1) * 512],
                        pp2[0:tsz, :],
                        pbb[0:tsz, oc * 512:(oc + 1) * 512],
                    )
                    if oc == 1:
                        nc.sync.dma_start(
                            out_ext[tt * 128:tt * 128 + tsz, :], ost[0:tsz, :]
                        )
                        b4_state["tt"] += 1
                        b4_state["oc"] = 0
                    else:
                        b4_state["oc"] = 1
                    return True

                # Software-pipelined attention: scores+softmax for unit i+1
                # are emitted before the rbp/PV consumers of unit i, so the
                # exp->mul latency hides behind the previous unit's matmuls.
                units = [(bp, hp, hh)
                         for bp in range(BC // 2)
                         for hp in range(8)
                         for hh in range(2)]
                ps_of, pt_of, po_of, rbp_of = {}, {}, {}, {}

                def emit_scores(u):
                    bp, hp, hh = u
                    qpo = 64 * hh
                    kot = 8 + hp
                    ps = ps_s.tile([128, 1024], F32, name="ps", tag="ps")
                    for kt, (ko, ksz, co) in enumerate(
                            ((0, KA, 0), (KA, KB, 512))):
                        for bi in range(2):
                            t0 = (2 * bp + bi) * NSEQ
                            nc.tensor.matmul(
                                ps[0:ksz,
                                   co + bi * NSEQ:co + (bi + 1) * NSEQ],
                                QKT[kot][qpo:qpo + 64,
                                         t0 + ko:t0 + ko + ksz],
                                QKT[hp][qpo:qpo + 64, t0:t0 + NSEQ],
                                start=True, stop=True,
                            )
                    h = 2 * hp + hh
                    pt = attw.tile([128, 1024], BF16, name="pt", tag="pt")
                    nc.scalar.activation(pt[:], ps[:], AF.Exp, scale=SCALE)
                    nc.vector.tensor_mul(
                        pt[0:KA, 0:W2], pt[0:KA, 0:W2],
                        EBA[:, h * W2:(h + 1) * W2],
                    )
                    nc.gpsimd.tensor_mul(
                        pt[0:KB, 512:512 + W2], pt[0:KB, 512:512 + W2],
                        EBB[:, h * W2:(h + 1) * W2],
                    )
                    pt_of[u] = pt

                def emit_use(u):
                    bp, hp, hh = u
                    h = 2 * hp + hh
                    cpos = 64 * hh
                    pt = pt_of.pop(u)
                    if hh == 0:
                        po_of[(bp, hp)] = ps_o.tile(
                            [128, W2], F32, name="po", tag="po",
                            padded_shape=[128, 512])
                        rbp_of[(bp, hp)] = ps_r.tile(
                            [128, W2], F32, name="rbp", tag="rbp",
                            padded_shape=[128, 512])
                    po = po_of[(bp, hp)]
                    rbp = rbp_of[(bp, hp)]
                    for kt, (ko, ksz, co) in enumerate(
                            ((0, KA, 0), (KA, KB, 512))):
                        nc.tensor.matmul(
                            rbp[cpos:cpos + 64, :],
                            onesK[0:ksz, :], pt[0:ksz, co:co + W2],
                            start=(kt == 0), stop=(kt == 1),
                            tile_position=(0, cpos),
                        )
                    for bi in range(2):
                        b = 2 * bp + bi
                        for kt, (ko, ksz, co) in enumerate(
                                ((0, KA, 0), (KA, KB, 512))):
                            nc.tensor.matmul(
                                po[cpos:cpos + 64,
                                   bi * NSEQ:(bi + 1) * NSEQ],
                                V1[(b, kt)][0:ksz, h * HD:(h + 1) * HD],
                                pt[0:ksz,
                                   co + bi * NSEQ:co + (bi + 1) * NSEQ],
                                start=(kt == 0), stop=(kt == 1),
                                tile_position=(0, cpos),
                            )
                    if hh == 1:
                        rbs = attsmall.tile([128, W2], F32,
                                            name="rbs", tag="rbs")
                        nc.vector.reciprocal_approx_fast(rbs[:], rbp[:])
                        nc.vector.tensor_mul(
                            attnT[hp][:, 2 * bp * NSEQ:2 * bp * NSEQ + W2],
                            po[:], rbs[:],
                        )
                        po_of.pop((bp, hp))
                        rbp_of.pop((bp, hp))

                emit_scores(units[0])
                for i, u in enumerate(units):
                    if i + 1 < len(units):
                        emit_scores(units[i + 1])
                    emit_b4_half((2 * u[0] * NSEQ) // 128)
                    emit_use(u)
                while emit_b4_half(NT):
                    pass

    return nc


_NC = None
LAST_RESULT = None


def _get_nc():
    global _NC
    if _NC is None:
        _NC = build_nc()
    return _NC


def _default_rel_pos_index():
    coords = np.stack(np.meshgrid(np.arange(WIN), np.arange(WIN), indexing='ij'))
    coords_flatten = coords.reshape(2, -1)
    rel = coords_flatten[:, :, None] - coords_flatten[:, None, :]
    rel = rel.transpose(1, 2, 0).astype(np.int64)
    rel[:, :, 0] += WIN - 1
    rel[:, :, 1] += WIN - 1
    rel[:, :, 0] *= 2 * WIN - 1
    idx = np.zeros((NSEQ, NSEQ), dtype=np.int64)
    idx[1:, 1:] = rel.sum(-1)
    idx[0, :] = NREL - 3
    idx[:, 0] = NREL - 2
    idx[0, 0] = NREL - 1
    return idx


def make_in_maps(x, qkv_w, q_bias, v_bias, rpb_table, proj_w, proj_b,
                 rel_pos_index=None):
    bf = ml_dtypes.bfloat16
    x = np.asarray(x, np.float32)
    wT = np.ascontiguousarray(np.asarray(qkv_w, np.float32).T.astype(bf))
    pwT = np.ascontiguousarray(np.asarray(proj_w, np.float32).T.astype(bf))
    qbc = np.ascontiguousarray(
        np.asarray(q_bias, np.float32).reshape(8, 128).T
    )
    vbb = np.ascontiguousarray(
        np.broadcast_to(np.asarray(v_bias, np.float32), (128, C)).astype(bf)
    )
    pbb = np.ascontiguousarray(
        np.broadcast_to(np.asarray(proj_b, np.float32), (128, C))
    )
    idx = (np.asarray(rel_pos_index) if rel_pos_index is not None
           else _default_rel_pos_index())
    bias = np.asarray(rpb_table, np.float32)[idx]          # [q, k, h]
    ebT = np.exp(bias).transpose(2, 1, 0)                  # [h, k, q]
    eb2 = np.concatenate([ebT, ebT], axis=2)               # [h, k, 2 batches q]
    eba = np.ascontiguousarray(
        eb2[:, :KA, :].transpose(1, 0, 2).reshape(KA, H * W2).astype(bf)
    )
    ebb = np.ascontiguousarray(
        eb2[:, KA:, :].transpose(1, 0, 2).reshape(KB, H * W2).astype(bf)
    )
    in_maps = []
    for c in range(8):
        xs = np.ascontiguousarray(
            x[c * BC:(c + 1) * BC].reshape(T, C).T.astype(bf)
        )
        in_maps.append({
            "xT": xs, "qkv_wT": wT, "proj_wT": pwT, "eba": eba, "ebb": ebb,
            "qb_col": qbc, "vb_bcast": vbb, "pb_bcast": pbb,
        })
    return in_maps


def _ensure_axon_hooks_importable():
    """bass_utils imports antenv.axon_hooks when BASS_TRACE is set; the image's
    antenv lacks that module. Provide a no-op stand-in so tracing degrades
    gracefully instead of crashing (unless a real one is already installed)."""
    import types
    try:
        import antenv.axon_hooks  # noqa: F401
    except Exception:
        mod = types.ModuleType("antenv.axon_hooks")
        mod._h = None
        mod.set_axon_ntff_profile_hook = lambda h: setattr(mod, "_h", h)
        mod.get_axon_ntff_profile_hook = lambda: mod._h
        sys.modules["antenv.axon_hooks"] = mod
        try:
            import antenv
            antenv.axon_hooks = mod
        except Exception:
            pass


def kernel(x, qkv_w, q_bias, v_bias, rpb_table, proj_w, proj_b,
           rel_pos_index=None, **_unused):
    global LAST_RESULT
    _ensure_axon_hooks_importable()
    from concourse.bass_utils import run_bass_kernel_spmd

    nc = _get_nc()
    in_maps = make_in_maps(x, qkv_w, q_bias, v_bias, rpb_table, proj_w, proj_b,
                           rel_pos_index)
    res = run_bass_kernel_spmd(nc, in_maps, core_ids=list(range(8)))
    LAST_RESULT = res
    out = np.concatenate(
        [res.results[c]["out"].reshape(BC, NSEQ, C) for c in range(8)], axis=0
    )
    return out.astype(np.float32)


# revision 10
# speedup vs baseline: 1.1528x; 1.0290x over previous
"""Trainium2 Bass kernel for ViT window attention with relative position bias.

Full inputs in, full outputs out. Data-parallel over batch: 64 batches split
8 per NeuronCore, weights replicated, no collectives.

All layout transforms (x/weight transposes, bf16 casts, rel-pos bias gather
and exp) happen on host in make_in_maps; the device graph is pure GEMMs plus
the softmax, so the tensor engine streams bf16 matmuls back to back.
"""

import os
import sys

for _p in ("/opt/trn_rl_repo", "/root/.axon_site/_ro/trn_rl_repo"):
    if os.path.isdir(_p) and _p not in sys.path:
        sys.path.insert(0, _p)

import numpy as np
import ml_dtypes

import concourse.bass as bass
import concourse.mybir as mybir
import concourse.tile as tile
from concourse import bacc
from concourse.bass import AP

F32 = mybir.dt.float32
BF16 = mybir.dt.bfloat16
AF = mybir.ActivationFunctionType

# problem constants
WIN = 14
NSEQ = WIN * WIN + 1          # 197
H = 16                        # heads
HD = 64                       # head dim
C = 1024
NREL = (2 * WIN - 1) * (2 * WIN - 1) + 3   # 732
B_FULL = 64
BC = 8                        # batches per core
T = BC * NSEQ                 # 1576 tokens per core
SCALE = HD ** -0.5            # 0.125
TCH = 394                     # qkv t-chunk (4 * 394 = 1576, fits one psum bank)
NT = 13                       # ceil(1576 / 128) output-projection token tiles
KA, KB = 128, 69              # key split per batch (197 = 128 + 69)
W2 = 2 * NSEQ                 # 394: two batches of queries per attention tile


def build_nc():
    nc = _build_graph()
    nc.compile()
    return nc


def _build_graph():
    nc = bacc.Bacc(None)

    xT_ext = nc.declare_dram_parameter("xT", [C, T], BF16, isOutput=False)
    wT_ext = nc.declare_dram_parameter("qkv_wT", [C, 3 * C], BF16, isOutput=False)
    pwT_ext = nc.declare_dram_parameter("proj_wT", [C, C], BF16, isOutput=False)
    eba_ext = nc.declare_dram_parameter("eba", [KA, H * W2], BF16, isOutput=False)
    ebb_ext = nc.declare_dram_parameter("ebb", [KB, H * W2], BF16, isOutput=False)
    qbc_ext = nc.declare_dram_parameter("qb_col", [128, 8], F32, isOutput=False)
    vbb_ext = nc.declare_dram_parameter("vb_bcast", [128, C], BF16, isOutput=False)
    pbb_ext = nc.declare_dram_parameter("pb_bcast", [128, C], F32, isOutput=False)
    out_ext = nc.declare_dram_parameter("out", [T, C], F32, isOutput=True)

    with tile.TileContext(nc) as tc:
        with tc.tile_pool(name="persist", bufs=1) as pp:
            onesK = pp.tile([128, HD], BF16, name="onesK", tag="onesK")
            nc.gpsimd.memset(onesK[:], 1.0)
            qb_col = pp.tile([128, 8], F32, name="qb_col", tag="qb_col")
            nc.sync.dma_start(qb_col[:], qbc_ext[:])
            vbb = pp.tile([128, C], BF16, name="vbb", tag="vbb")
            pbb = pp.tile([128, C], F32, name="pbb", tag="pbb")
            EBA = pp.tile([KA, H * W2], BF16, name="eba", tag="eba")
            EBB = pp.tile([KB, H * W2], BF16, name="ebb", tag="ebb")

            # persistent products of phase 1
            QKT = [
                pp.tile([128, T], BF16, name=f"qkt{ot}", tag=f"qkt{ot}")
                for ot in range(16)
            ]
            V1 = {}
            for b in range(BC):
                V1[(b, 0)] = pp.tile([KA, C], BF16, name=f"v1_{b}_0", tag=f"v1_{b}_0")
                V1[(b, 1)] = pp.tile([KB, C], BF16, name=f"v1_{b}_1", tag=f"v1_{b}_1")

            # ----- phase 1: QKV projection ---------------------------------
            with (
                tc.tile_pool(name="xw", bufs=1) as xw,
                tc.tile_pool(name="ps1", bufs=4, space="PSUM") as ps1,
            ):
                # Block-nested SBUF layouts: each DMA fills a contiguous,
                # disjoint column range so matmuls only wait on the one DMA
                # that wrote their operand (region deps stay tight).
                # xTall col(ct, t) = tci*3152 + ct*394 + t%394, tci = t//394
                # WTall col(ct, o) = og*4096 + ct*512 + o%512,  og = o//512
                xTall = xw.tile([128, 8 * T], BF16, name="xtall", tag="xtall")
                WTall = xw.tile([128, 8 * 3 * C], BF16, name="wtall", tag="wtall")

                def xs(ct, t0, tl):  # x^T slice: chunk ct, tokens [t0, t0+tl)
                    tci = t0 // TCH
                    assert (t0 - tci * TCH) + tl <= TCH
                    a = xTall[:]
                    return AP(a.tensor,
                              a.offset + tci * 8 * TCH + ct * TCH + t0 % TCH,
                              [a.ap[0], [1, tl]])

                def ws(ct, o0, ol):  # qkv_w^T slice: chunk ct, outputs o range
                    og = o0 // 512
                    assert (o0 - og * 512) + ol <= 512
                    a = WTall[:]
                    return AP(a.tensor,
                              a.offset + og * 4096 + ct * 512 + o0 % 512,
                              [a.ap[0], [1, ol]])

                xa = xTall[:]
                wa = WTall[:]

                def dma_stripe(eng, tci, c0, cn):
                    eng.dma_start(
                        AP(xa.tensor, xa.offset + tci * 8 * TCH + c0 * TCH,
                           [xa.ap[0], [1, cn * TCH]]),
                        AP(xT_ext, c0 * 128 * T + tci * TCH,
                           [[T, 128], [128 * T, cn], [1, TCH]]),
                    )

                def dma_wog(eng, og, c0, cn):
                    eng.dma_start(
                        AP(wa.tensor, wa.offset + og * 4096 + c0 * 512,
                           [wa.ap[0], [1, cn * 512]]),
                        AP(wT_ext, c0 * 128 * 3 * C + og * 512,
                           [[3 * C, 128], [128 * 3 * C, cn], [1, 512]]),
                    )

                # cold start: spread across the three DMA-issuing queues in
                # the order the anti-diagonal QK schedule consumes the blocks
                dma_wog(nc.scalar, 0, 0, 2)
                dma_stripe(nc.sync, 0, 0, 2)
                dma_wog(nc.gpsimd, 0, 4, 4)
                dma_wog(nc.scalar, 0, 2, 2)
                dma_stripe(nc.sync, 0, 2, 2)
                dma_stripe(nc.sync, 0, 4, 4)
                dma_wog(nc.scalar, 1, 0, 8)
                dma_wog(nc.gpsimd, 2, 0, 8)
                dma_stripe(nc.sync, 1, 0, 8)
                dma_stripe(nc.sync, 2, 0, 8)
                dma_wog(nc.sync, 3, 0, 8)
                dma_stripe(nc.sync, 3, 0, 8)
                dma_wog(nc.scalar, 4, 0, 8)
                dma_wog(nc.scalar, 5, 0, 8)
                nc.gpsimd.dma_start(EBA[:], eba_ext[:])
                nc.gpsimd.dma_start(EBB[:], ebb_ext[:])
                nc.sync.dma_start(vbb[:], vbb_ext[:])
                nc.sync.dma_start(pbb[:], pbb_ext[:])

                def qk_chunk(ot, tci):
                    t0 = tci * TCH
                    pq = ps1.tile([128, 512], F32, name="pq", tag="pq")
                    for ct in range(8):
                        nc.tensor.matmul(
                            pq[:, 0:TCH],
                            ws(ct, ot * 128, 128),
                            xs(ct, t0, TCH),
                            start=(ct == 0), stop=(ct == 7),
                        )
                    if ot < 8:
                        nc.scalar.add(
                            QKT[ot][:, t0:t0 + TCH], pq[:, 0:TCH],
                            qb_col[:, ot:ot + 1],
                        )
                    else:
                        nc.vector.tensor_copy(QKT[ot][:, t0:t0 + TCH], pq[:, 0:TCH])

                # anti-diagonal (og, stripe) order matches DMA arrivals
                diag = sorted(
                    ((og, tci) for og in range(4) for tci in range(4)),
                    key=lambda p: (p[0] + p[1], p[1]),
                )
                for og, tci in diag:
                    for ot in range(4 * og, 4 * og + 4):
                        qk_chunk(ot, tci)

                # V projection -> V1[(b,kt)] bf16 (+bias)
                for b in range(BC):
                    for kt, (ko, ksz) in enumerate(((0, KA), (KA, KB))):
                        t0 = b * NSEQ + ko
                        for oc in range(2):
                            pv = ps1.tile([128, 512], F32, name="pv", tag="pq")
                            for ct in range(8):
                                nc.tensor.matmul(
                                    pv[0:ksz, :],
                                    xs(ct, t0, ksz),
                                    ws(ct, 2 * C + oc * 512, 512),
                                    start=(ct == 0), stop=(ct == 7),
                                )
                            nc.vector.tensor_add(
                                V1[(b, kt)][0:ksz, oc * 512:(oc + 1) * 512],
                                pv[0:ksz, :],
                                vbb[0:ksz, oc * 512:(oc + 1) * 512],
                            )

            # ----- phase 2: attention + output projection -------------------
            with (
                tc.tile_pool(name="ab", bufs=1) as ab,
                tc.tile_pool(name="attw", bufs=8) as attw,
                tc.tile_pool(name="attsmall", bufs=6) as attsmall,
                tc.tile_pool(name="ostage", bufs=3) as op_,
                tc.tile_pool(name="ps_s", bufs=2, space="PSUM") as ps_s,
                tc.tile_pool(name="ps_o", bufs=2, space="PSUM") as ps_o,
                tc.tile_pool(name="ps_r", bufs=2, space="PSUM") as ps_r,
            ):
                attnT = [
                    ab.tile([128, T], BF16, name=f"at{ct}", tag=f"at{ct}")
                    for ct in range(8)
                ]
                # PWTall col(ct, o) = oc*4096 + ct*512 + o%512, oc = o//512
                PWTall = ab.tile([128, 8 * C], BF16, name="pwtall", tag="pwtall")
                pa = PWTall[:]
                for oc in range(2):
                    nc.sync.dma_start(
                        AP(pa.tensor, pa.offset + oc * 4096,
                           [pa.ap[0], [1, 4096]]),
                        AP(pwT_ext, oc * 512,
                           [[C, 128], [128 * C, 8], [1, 512]]),
                    )

                def pws(ct, o0, ol):
                    oc = o0 // 512
                    assert (o0 - oc * 512) + ol <= 512
                    a = PWTall[:]
                    return AP(a.tensor,
                              a.offset + oc * 4096 + ct * 512 + o0 % 512,
                              [a.ap[0], [1, ol]])

                # output projection, emitted one 512-col half at a time so it
                # can fill tensor-engine gaps inside the attention loop
                b4_state = {"tt": 0, "oc": 0, "ost": None}

                def b4_ready():
                    return b4_state["tt"] < NT

                def emit_b4_half(limit_tt):
                    tt, oc = b4_state["tt"], b4_state["oc"]
                    if tt >= min(NT, limit_tt):
                        return False
                    tsz = min(128, T - tt * 128)
                    if oc == 0:
                        b4_state["ost"] = op_.tile([128, C], F32, name="ost",
                                                   tag="ost")
                    ost = b4_state["ost"]
                    pp2 = ps_r.tile([128, 512], F32, name="ppj", tag="rbp")
                    for ct in range(8):
                        nc.tensor.matmul(
                            pp2[0:tsz, :],
                            attnT[ct][:, tt * 128:tt * 128 + tsz],
                            pws(ct, oc * 512, 512),
                            start=(ct == 0), stop=(ct == 7),
                        )
                    nc.vector.tensor_add(
                        ost[0:tsz, oc * 512:(oc + 1) * 512],
                        pp2[0:tsz, :],
                        pbb[0:tsz, oc * 512:(oc + 1) * 512],
                    )
                    if oc == 1:
                        nc.sync.dma_start(
                            out_ext[tt * 128:tt * 128 + tsz, :], ost[0:tsz, :]
                        )
                        b4_state["tt"] += 1
                        b4_state["oc"] = 0
                    else:
                        b4_state["oc"] = 1
                    return True

                # Software-pipelined attention: scores+softmax for unit i+1
                # are emitted before the rbp/PV consumers of unit i, so the
                # exp->mul latency hides behind the previous unit's matmuls.
                units = [(bp, hp, hh)
                         for bp in range(BC // 2)
                         for hp in range(8)
                         for hh in range(2)]
                ps_of, pt_of, po_of, rbp_of = {}, {}, {}, {}

                def emit_scores(u):
                    bp, hp, hh = u
                    qpo = 64 * hh
                    kot = 8 + hp
                    ps = ps_s.tile([128, 1024], F32, name="ps", tag="ps")
                    for kt, (ko, ksz, co) in enumerate(
                            ((0, KA, 0), (KA, KB, 512))):
                        for bi in range(2):
                            t0 = (2 * bp + bi) * NSEQ
                            nc.tensor.matmul(
                                ps[0:ksz,
                                   co + bi * NSEQ:co + (bi + 1) * NSEQ],
                                QKT[kot][qpo:qpo + 64,
                                         t0 + ko:t0 + ko + ksz],
                                QKT[hp][qpo:qpo + 64, t0:t0 + NSEQ],
                                start=True, stop=True,
                            )
                    h = 2 * hp + hh
                    pt = attw.tile([128, 1024], BF16, name="pt", tag="pt")
                    nc.scalar.activation(pt[:], ps[:], AF.Exp, scale=SCALE)
                    nc.vector.tensor_mul(
                        pt[0:KA, 0:W2], pt[0:KA, 0:W2],
                        EBA[:, h * W2:(h + 1) * W2],
                    )
                    nc.gpsimd.tensor_mul(
                        pt[0:KB, 512:512 + W2], pt[0:KB, 512:512 + W2],
                        EBB[:, h * W2:(h + 1) * W2],
                    )
                    pt_of[u] = pt

                def emit_use(u):
                    bp, hp, hh = u
                    h = 2 * hp + hh
                    cpos = 64 * hh
                    pt = pt_of.pop(u)
                    if hh == 0:
                        po_of[(bp, hp)] = ps_o.tile(
                            [128, W2], F32, name="po", tag="po",
                            padded_shape=[128, 512])
                        rbp_of[(bp, hp)] = ps_r.tile(
                            [128, W2], F32, name="rbp", tag="rbp",
                            padded_shape=[128, 512])
                    po = po_of[(bp, hp)]
                    rbp = rbp_of[(bp, hp)]
                    for kt, (ko, ksz, co) in enumerate(
                            ((0, KA, 0), (KA, KB, 512))):
                        nc.tensor.matmul(
                            rbp[cpos:cpos + 64, :],
                            onesK[0:ksz, :], pt[0:ksz, co:co + W2],
                            start=(kt == 0), stop=(kt == 1),
                            tile_position=(0, cpos),
                        )
                    for bi in range(2):
                        b = 2 * bp + bi
                        for kt, (ko, ksz, co) in enumerate(
                                ((0, KA, 0), (KA, KB, 512))):
                            nc.tensor.matmul(
                                po[cpos:cpos + 64,
                                   bi * NSEQ:(bi + 1) * NSEQ],
                                V1[(b, kt)][0:ksz, h * HD:(h + 1) * HD],
                                pt[0:ksz,
                                   co + bi * NSEQ:co + (bi + 1) * NSEQ],
                                start=(kt == 0), stop=(kt == 1),
                                tile_position=(0, cpos),
                            )
                    if hh == 1:
                        rbs = attsmall.tile([128, W2], F32,
                                            name="rbs", tag="rbs")
                        nc.vector.reciprocal_approx_fast(rbs[:], rbp[:])
                        nc.vector.tensor_mul(
                            attnT[hp][:, 2 * bp * NSEQ:2 * bp * NSEQ + W2],
                            po[:], rbs[:],
                        )
                        po_of.pop((bp, hp))
                        rbp_of.pop((bp, hp))

                emit_scores(units[0])
                emit_scores(units[1])
                for i, u in enumerate(units):
                    if i + 2 < len(units):
                        emit_scores(units[i + 2])
                    emit_b4_half((2 * u[0] * NSEQ) // 128)
                    emit_use(u)
                while emit_b4_half(NT):
                    pass

    return nc


_NC = None
LAST_RESULT = None


def _get_nc():
    global _NC
    if _NC is None:
        _NC = build_nc()
    return _NC


def _default_rel_pos_index():
    coords = np.stack(np.meshgrid(np.arange(WIN), np.arange(WIN), indexing='ij'))
    coords_flatten = coords.reshape(2, -1)
    rel = coords_flatten[:, :, None] - coords_flatten[:, None, :]
    rel = rel.transpose(1, 2, 0).astype(np.int64)
    rel[:, :, 0] += WIN - 1
    rel[:, :, 1] += WIN - 1
    rel[:, :, 0] *= 2 * WIN - 1
    idx = np.zeros((NSEQ, NSEQ), dtype=np.int64)
    idx[1:, 1:] = rel.sum(-1)
    idx[0, :] = NREL - 3
    idx[:, 0] = NREL - 2
    idx[0, 0] = NREL - 1
    return idx


def make_in_maps(x, qkv_w, q_bias, v_bias, rpb_table, proj_w, proj_b,
                 rel_pos_index=None):
    bf = ml_dtypes.bfloat16
    x = np.asarray(x, np.float32)
    wT = np.ascontiguousarray(np.asarray(qkv_w, np.float32).T.astype(bf))
    pwT = np.ascontiguousarray(np.asarray(proj_w, np.float32).T.astype(bf))
    qbc = np.ascontiguousarray(
        np.asarray(q_bias, np.float32).reshape(8, 128).T
    )
    vbb = np.ascontiguousarray(
        np.broadcast_to(np.asarray(v_bias, np.float32), (128, C)).astype(bf)
    )
    pbb = np.ascontiguousarray(
        np.broadcast_to(np.asarray(proj_b, np.float32), (128, C))
    )
    idx = (np.asarray(rel_pos_index) if rel_pos_index is not None
           else _default_rel_pos_index())
    bias = np.asarray(rpb_table, np.float32)[idx]          # [q, k, h]
    ebT = np.exp(bias).transpose(2, 1, 0)                  # [h, k, q]
    eb2 = np.concatenate([ebT, ebT], axis=2)               # [h, k, 2 batches q]
    eba = np.ascontiguousarray(
        eb2[:, :KA, :].transpose(1, 0, 2).reshape(KA, H * W2).astype(bf)
    )
    ebb = np.ascontiguousarray(
        eb2[:, KA:, :].transpose(1, 0, 2).reshape(KB, H * W2).astype(bf)
    )
    in_maps = []
    for c in range(8):
        xs = np.ascontiguousarray(
            x[c * BC:(c + 1) * BC].reshape(T, C).T.astype(bf)
        )
        in_maps.append({
            "xT": xs, "qkv_wT": wT, "proj_wT": pwT, "eba": eba, "ebb": ebb,
            "qb_col": qbc, "vb_bcast": vbb, "pb_bcast": pbb,
        })
    return in_maps


def _ensure_axon_hooks_importable():
    """bass_utils imports antenv.axon_hooks when BASS_TRACE is set; the image's
    antenv lacks that module. Provide a no-op stand-in so tracing degrades
    gracefully instead of crashing (unless a real one is already installed)."""
    import types
    try:
        import antenv.axon_hooks  # noqa: F401
    except Exception:
        mod = types.ModuleType("antenv.axon_hooks")
        mod._h = None
        mod.set_axon_ntff_profile_hook = lambda h: setattr(mod, "_h", h)
        mod.get_axon_ntff_profile_hook = lambda: mod._h
        sys.modules["antenv.axon_hooks"] = mod
        try:
            import antenv
            antenv.axon_hooks = mod
        except Exception:
            pass


def kernel(x, qkv_w, q_bias, v_bias, rpb_table, proj_w, proj_b,
           rel_pos_index=None, **_unused):
    global LAST_RESULT
    _ensure_axon_hooks_importable()
    from concourse.bass_utils import run_bass_kernel_spmd

    nc = _get_nc()
    in_maps = make_in_maps(x, qkv_w, q_bias, v_bias, rpb_table, proj_w, proj_b,
                           rel_pos_index)
    res = run_bass_kernel_spmd(nc, in_maps, core_ids=list(range(8)))
    LAST_RESULT = res
    out = np.concatenate(
        [res.results[c]["out"].reshape(BC, NSEQ, C) for c in range(8)], axis=0
    )
    return out.astype(np.float32)


# revision 11
# speedup vs baseline: 1.1827x; 1.0259x over previous
"""Trainium2 Bass kernel for ViT window attention with relative position bias.

Full inputs in, full outputs out. Data-parallel over batch: 64 batches split
8 per NeuronCore, weights replicated, no collectives.

All layout transforms (x/weight transposes, bf16 casts, rel-pos bias gather
and exp) happen on host in make_in_maps; the device graph is pure GEMMs plus
the softmax, so the tensor engine streams bf16 matmuls back to back.
"""

import os
import sys

for _p in ("/opt/trn_rl_repo", "/root/.axon_site/_ro/trn_rl_repo"):
    if os.path.isdir(_p) and _p not in sys.path:
        sys.path.insert(0, _p)

import numpy as np
import ml_dtypes

import concourse.bass as bass
import concourse.mybir as mybir
import concourse.tile as tile
from concourse import bacc
from concourse.bass import AP

F32 = mybir.dt.float32
BF16 = mybir.dt.bfloat16
AF = mybir.ActivationFunctionType

# problem constants
WIN = 14
NSEQ = WIN * WIN + 1          # 197
H = 16                        # heads
HD = 64                       # head dim
C = 1024
NREL = (2 * WIN - 1) * (2 * WIN - 1) + 3   # 732
B_FULL = 64
BC = 8                        # batches per core
T = BC * NSEQ                 # 1576 tokens per core
SCALE = HD ** -0.5            # 0.125
TCH = 394                     # qkv t-chunk (4 * 394 = 1576, fits one psum bank)
NT = 13                       # ceil(1576 / 128) output-projection token tiles
KA, KB = 128, 69              # key split per batch (197 = 128 + 69)
W2 = 2 * NSEQ                 # 394: two batches of queries per attention tile


def build_nc():
    nc = _build_graph()
    nc.compile()
    return nc


def _build_graph():
    nc = bacc.Bacc(None)

    xT_ext = nc.declare_dram_parameter("xT", [C, T], BF16, isOutput=False)
    wT_ext = nc.declare_dram_parameter("qkv_wT", [C, 3 * C], BF16, isOutput=False)
    pwT_ext = nc.declare_dram_parameter("proj_wT", [C, C], BF16, isOutput=False)
    eba_ext = nc.declare_dram_parameter("eba", [KA, H * W2], BF16, isOutput=False)
    ebb_ext = nc.declare_dram_parameter("ebb", [KB, H * W2], BF16, isOutput=False)
    qbc_ext = nc.declare_dram_parameter("qb_col", [128, 8], F32, isOutput=False)
    vbb_ext = nc.declare_dram_parameter("vb_bcast", [128, C], BF16, isOutput=False)
    pbb_ext = nc.declare_dram_parameter("pb_bcast", [128, C], F32, isOutput=False)
    out_ext = nc.declare_dram_parameter("out", [T, C], F32, isOutput=True)

    with tile.TileContext(nc) as tc:
        with tc.tile_pool(name="persist", bufs=1) as pp:
            onesK = pp.tile([128, HD], BF16, name="onesK", tag="onesK")
            nc.gpsimd.memset(onesK[:], 1.0)
            qb_col = pp.tile([128, 8], F32, name="qb_col", tag="qb_col")
            nc.sync.dma_start(qb_col[:], qbc_ext[:])
            vbb = pp.tile([128, C], BF16, name="vbb", tag="vbb")
            pbb = pp.tile([128, C], F32, name="pbb", tag="pbb")
            EBA = pp.tile([KA, H * W2], BF16, name="eba", tag="eba")
            EBB = pp.tile([KB, H * W2], BF16, name="ebb", tag="ebb")

            # persistent products of phase 1
            QKT = [
                pp.tile([128, T], BF16, name=f"qkt{ot}", tag=f"qkt{ot}")
                for ot in range(16)
            ]
            V1 = {}
            for b in range(BC):
                V1[(b, 0)] = pp.tile([KA, C], BF16, name=f"v1_{b}_0", tag=f"v1_{b}_0")
                V1[(b, 1)] = pp.tile([KB, C], BF16, name=f"v1_{b}_1", tag=f"v1_{b}_1")

            # ----- phase 1: QKV projection ---------------------------------
            with (
                tc.tile_pool(name="xw", bufs=1) as xw,
                tc.tile_pool(name="ps1", bufs=4, space="PSUM") as ps1,
            ):
                # Block-nested SBUF layouts: each DMA fills a contiguous,
                # disjoint column range so matmuls only wait on the one DMA
                # that wrote their operand (region deps stay tight).
                # xTall col(ct, t) = tci*3152 + ct*394 + t%394, tci = t//394
                # WTall col(ct, o) = og*4096 + ct*512 + o%512,  og = o//512
                xTall = xw.tile([128, 8 * T], BF16, name="xtall", tag="xtall")
                WTall = xw.tile([128, 8 * 3 * C], BF16, name="wtall", tag="wtall")

                def xs(ct, t0, tl):  # x^T slice: chunk ct, tokens [t0, t0+tl)
                    tci = t0 // TCH
                    assert (t0 - tci * TCH) + tl <= TCH
                    a = xTall[:]
                    return AP(a.tensor,
                              a.offset + tci * 8 * TCH + ct * TCH + t0 % TCH,
                              [a.ap[0], [1, tl]])

                def ws(ct, o0, ol):  # qkv_w^T slice: chunk ct, outputs o range
                    og = o0 // 512
                    assert (o0 - og * 512) + ol <= 512
                    a = WTall[:]
                    return AP(a.tensor,
                              a.offset + og * 4096 + ct * 512 + o0 % 512,
                              [a.ap[0], [1, ol]])

                xa = xTall[:]
                wa = WTall[:]

                def dma_stripe(eng, tci, c0, cn):
                    eng.dma_start(
                        AP(xa.tensor, xa.offset + tci * 8 * TCH + c0 * TCH,
                           [xa.ap[0], [1, cn * TCH]]),
                        AP(xT_ext, c0 * 128 * T + tci * TCH,
                           [[T, 128], [128 * T, cn], [1, TCH]]),
                    )

                def dma_wog(eng, og, c0, cn):
                    eng.dma_start(
                        AP(wa.tensor, wa.offset + og * 4096 + c0 * 512,
                           [wa.ap[0], [1, cn * 512]]),
                        AP(wT_ext, c0 * 128 * 3 * C + og * 512,
                           [[3 * C, 128], [128 * 3 * C, cn], [1, 512]]),
                    )

                # Big DMAs stall the sync/scalar sequencers (~15ns/descriptor
                # past the ring size) but issue in ~1us from gpsimd's SWDGE,
                # so gpsimd carries the bulk stream in consumption order while
                # sync/scalar push small quarter-DMAs to cut the cold start.
                dma_wog(nc.scalar, 0, 0, 2)
                dma_stripe(nc.sync, 0, 0, 2)
                dma_wog(nc.gpsimd, 0, 4, 4)
                dma_wog(nc.scalar, 0, 2, 2)
                dma_stripe(nc.sync, 0, 2, 2)
                dma_stripe(nc.gpsimd, 0, 4, 4)
                dma_wog(nc.gpsimd, 1, 0, 8)
                dma_stripe(nc.gpsimd, 1, 0, 8)
                dma_wog(nc.gpsimd, 2, 0, 8)
                dma_stripe(nc.gpsimd, 2, 0, 8)
                dma_wog(nc.gpsimd, 3, 0, 8)
                dma_stripe(nc.gpsimd, 3, 0, 8)
                dma_wog(nc.gpsimd, 4, 0, 8)
                dma_wog(nc.gpsimd, 5, 0, 8)
                nc.gpsimd.dma_start(EBA[:], eba_ext[:])
                nc.gpsimd.dma_start(EBB[:], ebb_ext[:])
                nc.sync.dma_start(vbb[:], vbb_ext[:])
                nc.sync.dma_start(pbb[:], pbb_ext[:])

                def qk_chunk(ot, tci):
                    t0 = tci * TCH
                    pq = ps1.tile([128, 512], F32, name="pq", tag="pq")
                    for ct in range(8):
                        nc.tensor.matmul(
                            pq[:, 0:TCH],
                            ws(ct, ot * 128, 128),
                            xs(ct, t0, TCH),
                            start=(ct == 0), stop=(ct == 7),
                        )
                    if ot < 8:
                        nc.scalar.add(
                            QKT[ot][:, t0:t0 + TCH], pq[:, 0:TCH],
                            qb_col[:, ot:ot + 1],
                        )
                    else:
                        nc.vector.tensor_copy(QKT[ot][:, t0:t0 + TCH], pq[:, 0:TCH])

                # anti-diagonal (og, stripe) order matches DMA arrivals
                diag = sorted(
                    ((og, tci) for og in range(4) for tci in range(4)),
                    key=lambda p: (p[0] + p[1], p[1]),
                )
                for og, tci in diag:
                    for ot in range(4 * og, 4 * og + 4):
                        qk_chunk(ot, tci)

                # V projection -> V1[(b,kt)] bf16 (+bias)
                for b in range(BC):
                    for kt, (ko, ksz) in enumerate(((0, KA), (KA, KB))):
                        t0 = b * NSEQ + ko
                        for oc in range(2):
                            pv = ps1.tile([128, 512], F32, name="pv", tag="pq")
                            for ct in range(8):
                                nc.tensor.matmul(
                                    pv[0:ksz, :],
                                    xs(ct, t0, ksz),
                                    ws(ct, 2 * C + oc * 512, 512),
                                    start=(ct == 0), stop=(ct == 7),
                                )
                            nc.vector.tensor_add(
                                V1[(b, kt)][0:ksz, oc * 512:(oc + 1) * 512],
                                pv[0:ksz, :],
                                vbb[0:ksz, oc * 512:(oc + 1) * 512],
                            )

            # ----- phase 2: attention + output projection -------------------
            with (
                tc.tile_pool(name="ab", bufs=1) as ab,
                tc.tile_pool(name="attw", bufs=8) as attw,
                tc.tile_pool(name="attsmall", bufs=6) as attsmall,
                tc.tile_pool(name="ostage", bufs=3) as op_,
                tc.tile_pool(name="ps_s", bufs=2, space="PSUM") as ps_s,
                tc.tile_pool(name="ps_o", bufs=2, space="PSUM") as ps_o,
                tc.tile_pool(name="ps_r", bufs=2, space="PSUM") as ps_r,
            ):
                attnT = [
                    ab.tile([128, T], BF16, name=f"at{ct}", tag=f"at{ct}")
                    for ct in range(8)
                ]
                # PWTall col(ct, o) = oc*4096 + ct*512 + o%512, oc = o//512
                PWTall = ab.tile([128, 8 * C], BF16, name="pwtall", tag="pwtall")
                pa = PWTall[:]
                for oc in range(2):
                    nc.gpsimd.dma_start(
                        AP(pa.tensor, pa.offset + oc * 4096,
                           [pa.ap[0], [1, 4096]]),
                        AP(pwT_ext, oc * 512,
                           [[C, 128], [128 * C, 8], [1, 512]]),
                    )

                def pws(ct, o0, ol):
                    oc = o0 // 512
                    assert (o0 - oc * 512) + ol <= 512
                    a = PWTall[:]
                    return AP(a.tensor,
                              a.offset + oc * 4096 + ct * 512 + o0 % 512,
                              [a.ap[0], [1, ol]])

                # output projection, emitted one 512-col half at a time so it
                # can fill tensor-engine gaps inside the attention loop
                b4_state = {"tt": 0, "oc": 0, "ost": None}

                def b4_ready():
                    return b4_state["tt"] < NT

                def emit_b4_half(limit_tt):
                    tt, oc = b4_state["tt"], b4_state["oc"]
                    if tt >= min(NT, limit_tt):
                        return False
                    tsz = min(128, T - tt * 128)
                    if oc == 0:
                        b4_state["ost"] = op_.tile([128, C], F32, name="ost",
                                                   tag="ost")
                    ost = b4_state["ost"]
                    pp2 = ps_r.tile([128, 512], F32, name="ppj", tag="rbp")
                    for ct in range(8):
                        nc.tensor.matmul(
                            pp2[0:tsz, :],
                            attnT[ct][:, tt * 128:tt * 128 + tsz],
                            pws(ct, oc * 512, 512),
                            start=(ct == 0), stop=(ct == 7),
                        )
                    nc.vector.tensor_add(
                        ost[0:tsz, oc * 512:(oc + 1) * 512],
                        pp2[0:tsz, :],
                        pbb[0:tsz, oc * 512:(oc + 1) * 512],
                    )
                    if oc == 1:
                        nc.sync.dma_start(
                            out_ext[tt * 128:tt * 128 + tsz, :], ost[0:tsz, :]
                        )
                        b4_state["tt"] += 1
                        b4_state["oc"] = 0
                    else:
                        b4_state["oc"] = 1
                    return True

                # Software-pipelined attention: scores+softmax for unit i+1
                # are emitted before the rbp/PV consumers of unit i, so the
                # exp->mul latency hides behind the previous unit's matmuls.
                units = [(bp, hp, hh)
                         for bp in range(BC // 2)
                         for hp in range(8)
                         for hh in range(2)]
                ps_of, pt_of, po_of, rbp_of = {}, {}, {}, {}

                def emit_scores(u):
                    bp, hp, hh = u
                    qpo = 64 * hh
                    kot = 8 + hp
                    ps = ps_s.tile([128, 1024], F32, name="ps", tag="ps")
                    for kt, (ko, ksz, co) in enumerate(
                            ((0, KA, 0), (KA, KB, 512))):
                        for bi in range(2):
                            t0 = (2 * bp + bi) * NSEQ
                            nc.tensor.matmul(
                                ps[0:ksz,
                                   co + bi * NSEQ:co + (bi + 1) * NSEQ],
                                QKT[kot][qpo:qpo + 64,
                                         t0 + ko:t0 + ko + ksz],
                                QKT[hp][qpo:qpo + 64, t0:t0 + NSEQ],
                                start=True, stop=True,
                            )
                    h = 2 * hp + hh
                    pt = attw.tile([128, 1024], BF16, name="pt", tag="pt")
                    nc.scalar.activation(pt[:], ps[:], AF.Exp, scale=SCALE)
                    nc.vector.tensor_mul(
                        pt[0:KA, 0:W2], pt[0:KA, 0:W2],
                        EBA[:, h * W2:(h + 1) * W2],
                    )
                    nc.gpsimd.tensor_mul(
                        pt[0:KB, 512:512 + W2], pt[0:KB, 512:512 + W2],
                        EBB[:, h * W2:(h + 1) * W2],
                    )
                    pt_of[u] = pt

                def emit_use(u):
                    bp, hp, hh = u
                    h = 2 * hp + hh
                    cpos = 64 * hh
                    pt = pt_of.pop(u)
                    if hh == 0:
                        po_of[(bp, hp)] = ps_o.tile(
                            [128, W2], F32, name="po", tag="po",
                            padded_shape=[128, 512])
                        rbp_of[(bp, hp)] = ps_r.tile(
                            [128, W2], F32, name="rbp", tag="rbp",
                            padded_shape=[128, 512])
                    po = po_of[(bp, hp)]
                    rbp = rbp_of[(bp, hp)]
                    for kt, (ko, ksz, co) in enumerate(
                            ((0, KA, 0), (KA, KB, 512))):
                        nc.tensor.matmul(
                            rbp[cpos:cpos + 64, :],
                            onesK[0:ksz, :], pt[0:ksz, co:co + W2],
                            start=(kt == 0), stop=(kt == 1),
                            tile_position=(0, cpos),
                        )
                    for bi in range(2):
                        b = 2 * bp + bi
                        for kt, (ko, ksz, co) in enumerate(
                                ((0, KA, 0), (KA, KB, 512))):
                            nc.tensor.matmul(
                                po[cpos:cpos + 64,
                                   bi * NSEQ:(bi + 1) * NSEQ],
                                V1[(b, kt)][0:ksz, h * HD:(h + 1) * HD],
                                pt[0:ksz,
                                   co + bi * NSEQ:co + (bi + 1) * NSEQ],
                                start=(kt == 0), stop=(kt == 1),
                                tile_position=(0, cpos),
                            )
                    if hh == 1:
                        rbs = attsmall.tile([128, W2], F32,
                                            name="rbs", tag="rbs")
                        nc.vector.reciprocal_approx_fast(rbs[:], rbp[:])
                        nc.vector.tensor_mul(
                            attnT[hp][:, 2 * bp * NSEQ:2 * bp * NSEQ + W2],
                            po[:], rbs[:],
                        )
                        po_of.pop((bp, hp))
                        rbp_of.pop((bp, hp))

                emit_scores(units[0])
                emit_scores(units[1])
                for i, u in enumerate(units):
                    if i + 2 < len(units):
                        emit_scores(units[i + 2])
                    emit_b4_half((2 * u[0] * NSEQ) // 128)
                    emit_use(u)
                while emit_b4_half(NT):
                    pass

    return nc


_NC = None
LAST_RESULT = None


def _get_nc():
    global _NC
    if _NC is None:
        _NC = build_nc()
    return _NC


def _default_rel_pos_index():
    coords = np.stack(np.meshgrid(np.arange(WIN), np.arange(WIN), indexing='ij'))
    coords_flatten = coords.reshape(2, -1)
    rel = coords_flatten[:, :, None] - coords_flatten[:, None, :]
    rel = rel.transpose(1, 2, 0).astype(np.int64)
    rel[:, :, 0] += WIN - 1
    rel[:, :, 1] += WIN - 1
    rel[:, :, 0] *= 2 * WIN - 1
    idx = np.zeros((NSEQ, NSEQ), dtype=np.int64)
    idx[1:, 1:] = rel.sum(-1)
    idx[0, :] = NREL - 3
    idx[:, 0] = NREL - 2
    idx[0, 0] = NREL - 1
    return idx


def make_in_maps(x, qkv_w, q_bias, v_bias, rpb_table, proj_w, proj_b,
                 rel_pos_index=None):
    bf = ml_dtypes.bfloat16
    x = np.asarray(x, np.float32)
    wT = np.ascontiguousarray(np.asarray(qkv_w, np.float32).T.astype(bf))
    pwT = np.ascontiguousarray(np.asarray(proj_w, np.float32).T.astype(bf))
    qbc = np.ascontiguousarray(
        np.asarray(q_bias, np.float32).reshape(8, 128).T
    )
    vbb = np.ascontiguousarray(
        np.broadcast_to(np.asarray(v_bias, np.float32), (128, C)).astype(bf)
    )
    pbb = np.ascontiguousarray(
        np.broadcast_to(np.asarray(proj_b, np.float32), (128, C))
    )
    idx = (np.asarray(rel_pos_index) if rel_pos_index is not None
           else _default_rel_pos_index())
    bias = np.asarray(rpb_table, np.float32)[idx]          # [q, k, h]
    ebT = np.exp(bias).transpose(2, 1, 0)                  # [h, k, q]
    eb2 = np.concatenate([ebT, ebT], axis=2)               # [h, k, 2 batches q]
    eba = np.ascontiguousarray(
        eb2[:, :KA, :].transpose(1, 0, 2).reshape(KA, H * W2).astype(bf)
    )
    ebb = np.ascontiguousarray(
        eb2[:, KA:, :].transpose(1, 0, 2).reshape(KB, H * W2).astype(bf)
    )
    in_maps = []
    for c in range(8):
        xs = np.ascontiguousarray(
            x[c * BC:(c + 1) * BC].reshape(T, C).T.astype(bf)
        )
        in_maps.append({
            "xT": xs, "qkv_wT": wT, "proj_wT": pwT, "eba": eba, "ebb": ebb,
            "qb_col": qbc, "vb_bcast": vbb, "pb_bcast": pbb,
        })
    return in_maps


def _ensure_axon_hooks_importable():
    """bass_utils imports antenv.axon_hooks when BASS_TRACE is set; the image's
    antenv lacks that module. Provide a no-op stand-in so tracing degrades
    gracefully instead of crashing (unless a real one is already installed)."""
    import types
    try:
        import antenv.axon_hooks  # noqa: F401
    except Exception:
        mod = types.ModuleType("antenv.axon_hooks")
        mod._h = None
        mod.set_axon_ntff_profile_hook = lambda h: setattr(mod, "_h", h)
        mod.get_axon_ntff_profile_hook = lambda: mod._h
        sys.modules["antenv.axon_hooks"] = mod
        try:
            import antenv
            antenv.axon_hooks = mod
        except Exception:
            pass


def kernel(x, qkv_w, q_bias, v_bias, rpb_table, proj_w, proj_b,
           rel_pos_index=None, **_unused):
    global LAST_RESULT
    _ensure_axon_hooks_importable()
    from concourse.bass_utils import run_bass_kernel_spmd

    nc = _get_nc()
    in_maps = make_in_maps(x, qkv_w, q_bias, v_bias, rpb_table, proj_w, proj_b,
                           rel_pos_index)
    res = run_bass_kernel_spmd(nc, in_maps, core_ids=list(range(8)))
    LAST_RESULT = res
    out = np.concatenate(
        [res.results[c]["out"].reshape(BC, NSEQ, C) for c in range(8)], axis=0
    )
    return out.astype(np.float32)
